# revision 10
# baseline (speedup 1.0000x reference)
"""Trainium2 Bass kernel for single-head attention with row-major K-reshape.

Reference computation (per batch b):
    Q = x @ W_Q.T ; K = x @ W_K.T ; V = x @ W_V.T          # [S, D]
    K_r = K.reshape(D, S)          # row-major reshape, NOT a transpose
    scores = Q @ K_r / D
    out = softmax(scores, -1) @ V
Shapes: B=4, S=2048, D=1024, f32.

Sharding: 8 cores = (batch b in 0..3) x (pair-rank h in 0..1).  Core (b, h)
computes out[b, h*QB:(h+1)*QB, :].  K_r / V are built cooperatively by the
pair and exchanged with 2-rank AllGathers (with S == 2*D the row-major
reshape gives K_r[m, g*D + c] = K[2m + g, c], so rank g's K_r half is
x[g::2] @ W_K.T and its V half is its own query rows xq @ W_V.T).

Numerics: all five big matmuls run in fp8(e4m3) with DoubleRow perf mode
(2 contraction rows / PE cycle).  fp8 is safe here because:
  - scores are tiny (std ~1/32), so fp8 Q/K error feeds softmax as a ~1e-3
    absolute score perturbation -> negligible.
  - E = exp(scores) ~= 1, so we materialize e = E - 1 (small, fp8-safe) and
    use  softmax @ V = (colsum(V) + e.T @ V) / rsum,  with colsum(V) =
    (sum_rows x) @ W_V.T computed separately in bf16, folded in on the DVE
    drain, and rsum = S + rowsum(e).
  - weights are pre-scaled by 32 on the host so their elements (~N(0,1/D))
    land in fp8's normal range; the scale is folded into the exp scale and
    the final reciprocal.

The host passes pre-transposed operands (xT slices, W.T) so the device does
zero transposes of the inputs (PE transposes measured ~13x theoretical cost).

Per-core matmul dataflow (TensorE: out[M,N] = lhsT[K,M].T @ rhs[K,N],
contraction over the partition dim; all operand tiles are 3D
[P, k_tiles, cols] so DoubleRow can consume k-tile pairs):
    QT[m, i]     = lhsT=wqT[:, kk, m],  rhs=xqT[:, kk, i]     (fp8 DR)
    KRfrag[m, c] = lhsT=xpT[:, kk, m],  rhs=wkT[:, kk, c]     (fp8 DR)
    Vfrag[s', c] = lhsT=xqT[:, kk, s'], rhs=wvT[:, kk, c]     (fp8 DR)
    KR / V       = pair AllGather of fragments (DRAM bounce, fp8)
    ST[j, i]     = lhsT=KR[:, kk, j],   rhs=QT[:, kk, i]      (fp8 DR)
    Etmp         = exp(ST * 2^-20)            (ACT, psum->sbuf f32)
    ET           = Etmp - 1 -> fp8            (DVE)
    rsum[1, i]   = lhsT=ones, rhs=ET[:, kk, i]                (fp8 DR)
    O[i, c]      = lhsT=ET[:, kk, i], rhs=V[:, kk, c]         (fp8 DR)
    out          = (O + colsum_bcast) * (1 / (65536 + 32*rsum))  (DVE)
"""

from contextlib import ExitStack

import ml_dtypes
import numpy as np

import concourse.tile as tile
from concourse import bacc, mybir
from concourse.bass_utils import run_bass_kernel_spmd
from concourse.masks import make_identity

F32 = mybir.dt.float32
BF16 = mybir.dt.bfloat16
F8 = mybir.dt.float8e4
P = 128
DR = mybir.MatmulPerfMode.DoubleRow

NP_F8 = mybir.dt.np(F8)
NP_BF16 = mybir.dt.np(BF16)


def build_attention(nc, S=2048, D=1024, QB=1024, n_cores=8):
    """Emit the per-core attention program into `nc`. Requires S == 2*D == 2*QB."""
    assert S == 2 * D and QB == D and D % P == 0
    NST = S // P        # seq tiles (16)
    NDT = D // P        # d_model tiles (8)
    NQT = QB // P       # query tiles for this core (8)
    NC = 512            # matmul free-dim chunk (one PSUM bank of f32)
    NCH_D = D // NC     # chunks over output channels (2)
    NCH_Q = QB // NC    # chunks over queries (2)
    EXP = mybir.ActivationFunctionType.Exp
    groups = [[2 * b, 2 * b + 1] for b in range(n_cores // 2)]

    xqT_ap = nc.dram_tensor("xqT", [D, QB], F8, kind="ExternalInput").ap()
    xpT_ap = nc.dram_tensor("xpT", [D, D], F8, kind="ExternalInput").ap()
    wqT_ap = nc.dram_tensor("wqT", [D, D], F8, kind="ExternalInput").ap()
    wkT_ap = nc.dram_tensor("wkT", [D, D], F8, kind="ExternalInput").ap()
    wvT_ap = nc.dram_tensor("wvT", [D, D], F8, kind="ExternalInput").ap()
    wvT16_ap = nc.dram_tensor("wvT16", [D, D], BF16, kind="ExternalInput").ap()
    xsT_ap = nc.dram_tensor("xsT", [D, 1], BF16, kind="ExternalInput").ap()
    out_ap = nc.dram_tensor("out", [QB, D], BF16, kind="ExternalOutput").ap()

    with tile.TileContext(nc) as tc, ExitStack() as ctx:
        const_pool = ctx.enter_context(tc.tile_pool(name="const", bufs=1))
        big_pool = ctx.enter_context(tc.tile_pool(name="big", bufs=1))
        dram = ctx.enter_context(tc.tile_pool(name="dram", bufs=1, space="DRAM"))
        psum_mm = ctx.enter_context(tc.tile_pool(name="psum_mm", bufs=4, space="PSUM"))

        ones8 = const_pool.tile([P, NST, 1], F8)
        nc.vector.memset(ones8, 1.0)
        ones16 = const_pool.tile([1, P], BF16)
        nc.vector.memset(ones16, 1.0)
        identity = const_pool.tile([P, P], BF16)
        make_identity(nc, identity)

        # big operand tiles, 3D [P, k_tiles, cols]
        xqT = big_pool.tile([P, NDT, QB], F8, name="xqT_t")
        xpT = big_pool.tile([P, NDT, D], F8, name="xpT_t")
        wqT = big_pool.tile([P, NDT, D], F8, name="wqT_t")
        wkT = big_pool.tile([P, NDT, D], F8, name="wkT_t")
        wvT = big_pool.tile([P, NDT, D], F8, name="wvT_t")
        wvT16 = big_pool.tile([P, NDT, D], BF16, name="wvT16_t")
        xsT = big_pool.tile([P, NDT, 1], BF16, name="xsT_t")
        QT = big_pool.tile([P, NDT, QB], F8, name="QT_t")
        KR = big_pool.tile([P, NDT, S], F8, name="KR_t")
        V = big_pool.tile([P, NST, D], F8, name="V_t")
        ET = big_pool.tile([P, NST, QB], F8, name="ET_t")
        cbc = big_pool.tile([P, D], F32, name="cbc")        # colsum' broadcast
        csrow = big_pool.tile([1, D], BF16, name="csrow")   # colsum' row
        rsrow = big_pool.tile([1, QB], BF16, name="rsrow")  # rowsum(e) row
        rc_all = big_pool.tile([P, NQT], F32, name="rc_all")

        # DRAM bounce buffers for the pair AllGathers
        warm_in = dram.tile([1, P], F8, name="warm_in")
        warm_out = dram.tile([2, P], F8, name="warm_out")
        kr_frag = dram.tile([NDT, P, D], F8, name="kr_frag")
        kr_gath = dram.tile([2, NDT, P, D], F8, name="kr_gath")
        v_frag = dram.tile([NQT, P, D], F8, name="v_frag")
        v_gath = dram.tile([2, NQT, P, D], F8, name="v_gath")

        def load3d(dst3, src_ap, nrt, queue=nc.sync):
            for rt in range(nrt):
                queue.dma_start(
                    out=dst3[:, rt], in_=src_ap[rt * P:(rt + 1) * P, :]
                )

        def mm_chain(pm, lhsT3, lslice, rhs3, rslice, nkt):
            for kt in range(0, nkt, 2):
                nc.tensor.matmul(
                    pm[:],
                    lhsT3[:, kt:kt + 2, lslice],
                    rhs3[:, kt:kt + 2, rslice],
                    start=(kt == 0), stop=(kt == nkt - 2),
                    perf_mode=DR,
                )

        with tc.tile_pool(name="frag", bufs=2) as frag_pool, \
                tc.tile_pool(name="psum_cs", bufs=2, space="PSUM") as psum_cs:

            # ---- Phase A: K_r fragment -> AllGather (launch ASAP) ----
            # tiny warmup collective: absorbs the one-time global barrier and
            # CC-stream setup while the first loads are in flight
            nc.gpsimd.collective_compute(
                "AllGather", mybir.AluOpType.bypass, replica_groups=groups,
                ins=[warm_in.opt()], outs=[warm_out.opt()],
            )
            load3d(wkT, wkT_ap, NDT, queue=nc.sync)
            load3d(xpT, xpT_ap, NDT, queue=nc.scalar)
            for mt in range(NDT):
                kf = frag_pool.tile([P, D], F8, tag="kf", name="kf")
                for cch in range(NCH_D):
                    pm = psum_mm.tile([P, NC], F32, tag="pm")
                    mm_chain(pm, xpT, slice(mt * P, (mt + 1) * P),
                             wkT, slice(cch * NC, (cch + 1) * NC), NDT)
                    nc.scalar.copy(kf[:, cch * NC:(cch + 1) * NC], pm[:])
                (nc.scalar if mt % 2 == 0 else nc.gpsimd).dma_start(
                    out=kr_frag[mt], in_=kf[:])
            nc.gpsimd.collective_compute(
                "AllGather", mybir.AluOpType.bypass, replica_groups=groups,
                ins=[kr_frag.opt()], outs=[kr_gath.opt()],
            )

            # ---- Phase B: QT, V fragment, colsum (hides KR AllGather) ----
            load3d(xqT, xqT_ap, NDT, queue=nc.sync)
            load3d(wqT, wqT_ap, NDT, queue=nc.scalar)
            load3d(wvT, wvT_ap, NDT, queue=nc.gpsimd)
            load3d(wvT16, wvT16_ap, NDT, queue=nc.scalar)
            for kt in range(NDT):
                nc.gpsimd.dma_start(out=xsT[:, kt], in_=xsT_ap[kt * P:(kt + 1) * P, :])

            for mt in range(NDT):
                for ich in range(NCH_Q):
                    pm = psum_mm.tile([P, NC], F32, tag="pm")
                    mm_chain(pm, wqT, slice(mt * P, (mt + 1) * P),
                             xqT, slice(ich * NC, (ich + 1) * NC), NDT)
                    nc.vector.tensor_copy(QT[:, mt, ich * NC:(ich + 1) * NC], pm[:])

            for st in range(NQT):
                vf = frag_pool.tile([P, D], F8, tag="vf", name="vf")
                for cch in range(NCH_D):
                    pm = psum_mm.tile([P, NC], F32, tag="pm")
                    mm_chain(pm, xqT, slice(st * P, (st + 1) * P),
                             wvT, slice(cch * NC, (cch + 1) * NC), NDT)
                    nc.vector.tensor_copy(vf[:, cch * NC:(cch + 1) * NC], pm[:])
                nc.gpsimd.dma_start(out=v_frag[st], in_=vf[:])

            # colsum' = xsum @ (32 W_V).T, bf16 (accuracy-critical rank-1 term)
            for cch in range(NCH_D):
                pc = psum_cs.tile([1, NC], F32, tag="pc")
                for kt in range(NDT):
                    nc.tensor.matmul(
                        pc[:], xsT[:, kt, :], wvT16[:, kt, cch * NC:(cch + 1) * NC],
                        start=(kt == 0), stop=(kt == NDT - 1),
                    )
                nc.vector.tensor_copy(csrow[:, cch * NC:(cch + 1) * NC], pc[:])

            nc.gpsimd.collective_compute(
                "AllGather", mybir.AluOpType.bypass, replica_groups=groups,
                ins=[v_frag.opt()], outs=[v_gath.opt()],
            )

            # broadcast colsum' across partitions via K=1 ones matmul
            for cch in range(NCH_D):
                pm = psum_mm.tile([P, NC], F32, tag="pm")
                nc.tensor.matmul(
                    pm[:], ones16[:], csrow[:, cch * NC:(cch + 1) * NC],
                    start=True, stop=True,
                )
                nc.vector.tensor_copy(cbc[:, cch * NC:(cch + 1) * NC], pm[:])

            # pull gathered K_r halves into SBUF (ACT HWDGE queue is idle now)
            for g in range(2):
                for mt in range(NDT):
                    (nc.scalar if g == 0 else nc.gpsimd).dma_start(
                        out=KR[:, mt, g * D:(g + 1) * D], in_=kr_gath[g, mt]
                    )

        # ---- Phase C: scores + exp + e-cast; rsum interleaved ----
        with tc.tile_pool(name="estage", bufs=4) as estage, \
                tc.tile_pool(name="ostage", bufs=4) as ostage, \
                tc.tile_pool(name="psum_rs", bufs=1, space="PSUM") as psum_rs, \
                tc.tile_pool(name="psum_t", bufs=1, space="PSUM") as psum_t:

            # V loads (sync queue; behind the V AllGather)
            for g in range(2):
                for st in range(NQT):
                    (nc.sync if g == 0 else nc.gpsimd).dma_start(
                        out=V[:, g * NQT + st, :], in_=v_gath[g, st])

            prs = [psum_rs.tile([1, NC], F32, tag=f"prs{i}", name=f"prs{i}")
                   for i in range(NCH_Q)]
            for jt in range(NST):
                for ich in range(NCH_Q):
                    pm = psum_mm.tile([P, NC], F32, tag="pm")
                    mm_chain(pm, KR, slice(jt * P, (jt + 1) * P),
                             QT, slice(ich * NC, (ich + 1) * NC), NDT)
                    et = estage.tile([P, NC], F32, tag="et", name="et")
                    nc.scalar.activation(et[:], pm[:], EXP, scale=1.0 / (1 << 20))
                    nc.vector.tensor_scalar_add(
                        ET[:, jt, ich * NC:(ich + 1) * NC], et[:], -1.0
                    )
                # rsum accumulation (plain fp8 mode: DoubleRow with a
                # 1-partition output fails walrus codegen)
                for ich in range(NCH_Q):
                    nc.tensor.matmul(
                        prs[ich][:],
                        ones8[:, jt, :],
                        ET[:, jt, ich * NC:(ich + 1) * NC],
                        start=(jt == 0), stop=(jt == NST - 1),
                    )

            for ich in range(NCH_Q):
                nc.vector.tensor_copy(rsrow[:, ich * NC:(ich + 1) * NC], prs[ich][:])

            # ---- Phase D: recips + e.T @ V + combine ----
            # "transpose" rsrow [1, QB] into per-partition column [P, NQT] via
            # K=1 matmuls against a [1, 1] ones tile (lhsT free -> partitions)
            rs_t = psum_t.tile([P, NQT], F32, name="rs_t")
            for it in range(NQT):
                nc.tensor.matmul(
                    rs_t[:, it:it + 1], rsrow[:, it * P:(it + 1) * P],
                    ones16[:, 0:1], start=True, stop=True,
                )
            rs32 = const_pool.tile([P, NQT], F32, name="rs32")
            nc.vector.tensor_scalar(
                rs32[:], rs_t[:], 32.0, 65536.0,
                mybir.AluOpType.mult, mybir.AluOpType.add,
            )
            nc.vector.reciprocal(rc_all[:], rs32[:])

            for it in range(NQT):
                for cch in range(NCH_D):
                    pm = psum_mm.tile([P, NC], F32, tag="pm")
                    mm_chain(pm, ET, slice(it * P, (it + 1) * P),
                             V, slice(cch * NC, (cch + 1) * NC), NST)
                    ob = ostage.tile([P, NC], F32, tag="ob", name="ob")
                    nc.vector.tensor_add(ob[:], pm[:], cbc[:, cch * NC:(cch + 1) * NC])
                    ob16 = ostage.tile([P, NC], BF16, tag="ob16", name="ob16")
                    nc.vector.tensor_scalar_mul(ob16[:], ob[:], rc_all[:, it:it + 1])
                    (nc.sync if (it + cch) % 2 == 0 else nc.scalar).dma_start(
                        out=out_ap[it * P:(it + 1) * P, cch * NC:(cch + 1) * NC],
                        in_=ob16[:],
                    )
    return nc


_CACHE = {}


def _get_nc(S=2048, D=1024, QB=1024):
    key = (S, D, QB)
    if key not in _CACHE:
        nc = bacc.Bacc("TRN2", target_bir_lowering=False, debug=False, num_devices=8)
        build_attention(nc, S=S, D=D, QB=QB, n_cores=8)
        nc.compile()
        _CACHE[key] = nc
    return _CACHE[key]


def _run(x, W_Q, W_K, W_V, **spmd_kwargs):
    B, S, D = x.shape  # (4, 2048, 1024)
    QB = S // 2        # queries per core (1024)
    x = np.asarray(x, dtype=np.float32)
    W_Q = np.asarray(W_Q, dtype=np.float32)
    W_K = np.asarray(W_K, dtype=np.float32)
    W_V = np.asarray(W_V, dtype=np.float32)

    # weights pre-scaled by 32 so elements (~N(0, 1/D)) use fp8's normal range
    wqT8 = np.ascontiguousarray(W_Q.T * 32.0).astype(NP_F8)
    wkT8 = np.ascontiguousarray(W_K.T * 32.0).astype(NP_F8)
    wvT8 = np.ascontiguousarray(W_V.T * 32.0).astype(NP_F8)
    wvT16 = np.ascontiguousarray(W_V.T * 32.0).astype(NP_BF16)

    nc = _get_nc(S=S, D=D, QB=QB)
    in_maps = []
    for core in range(8):
        b, h = core // 2, core % 2
        xTb = x[b].T  # [D, S] view
        in_maps.append({
            "xqT": np.ascontiguousarray(xTb[:, h * QB:(h + 1) * QB]).astype(NP_F8),
            "xpT": np.ascontiguousarray(xTb[:, h::2]).astype(NP_F8),
            "wqT": wqT8, "wkT": wkT8, "wvT": wvT8, "wvT16": wvT16,
            "xsT": x[b].sum(axis=0, dtype=np.float64).astype(NP_BF16).reshape(D, 1),
        })
    res = run_bass_kernel_spmd(nc, in_maps, list(range(8)), **spmd_kwargs)
    out = np.empty((B, S, D), dtype=np.float32)
    for core in range(8):
        b, h = core // 2, core % 2
        out[b, h * QB:(h + 1) * QB, :] = np.asarray(
            res.results[core]["out"]).astype(np.float32)
    return out, res


def kernel(x, W_Q, W_K, W_V):
    return _run(x, W_Q, W_K, W_V)[0]


# revision 11
# speedup vs baseline: 1.1062x; 1.1062x over previous
"""Trainium2 Bass kernel for single-head attention with row-major K-reshape.

Reference computation (per batch b):
    Q = x @ W_Q.T ; K = x @ W_K.T ; V = x @ W_V.T          # [S, D]
    K_r = K.reshape(D, S)          # row-major reshape, NOT a transpose
    scores = Q @ K_r / D
    out = softmax(scores, -1) @ V
Shapes: B=4, S=2048, D=1024, f32.

Sharding: 8 cores = (batch b in 0..3) x (pair-rank h in 0..1).  Core (b, h)
computes out[b, h*QB:(h+1)*QB, :].  K_r / V are built cooperatively by the
pair and exchanged with 2-rank AllGathers (with S == 2*D the row-major
reshape gives K_r[m, g*D + c] = K[2m + g, c], so rank g's K_r half is
x[g::2] @ W_K.T and its V half is its own query rows xq @ W_V.T).

Numerics: all five big matmuls run in fp8(e4m3) with DoubleRow perf mode
(2 contraction rows / PE cycle).  fp8 is safe here because:
  - scores are tiny (std ~1/32), so fp8 Q/K error feeds softmax as a ~1e-3
    absolute score perturbation -> negligible.
  - E = exp(scores) ~= 1, so we materialize e = E - 1 (small, fp8-safe) and
    use  softmax @ V = (colsum(V) + e.T @ V) / rsum,  with colsum(V) =
    (sum_rows x) @ W_V.T computed separately in bf16, folded in on the DVE
    drain, and rsum = S + rowsum(e).
  - weights are pre-scaled by 32 on the host so their elements (~N(0,1/D))
    land in fp8's normal range; the scale is folded into the exp scale and
    the final reciprocal.

The host passes pre-transposed operands (xT slices, W.T) so the device does
zero transposes of the inputs (PE transposes measured ~13x theoretical cost).

Per-core matmul dataflow (TensorE: out[M,N] = lhsT[K,M].T @ rhs[K,N],
contraction over the partition dim; all operand tiles are 3D
[P, k_tiles, cols] so DoubleRow can consume k-tile pairs):
    QT[m, i]     = lhsT=wqT[:, kk, m],  rhs=xqT[:, kk, i]     (fp8 DR)
    KRfrag[m, c] = lhsT=xpT[:, kk, m],  rhs=wkT[:, kk, c]     (fp8 DR)
    Vfrag[s', c] = lhsT=xqT[:, kk, s'], rhs=wvT[:, kk, c]     (fp8 DR)
    KR / V       = pair AllGather of fragments (DRAM bounce, fp8)
    ST[j, i]     = lhsT=KR[:, kk, j],   rhs=QT[:, kk, i]      (fp8 DR)
    Etmp         = exp(ST * 2^-20)            (ACT, psum->sbuf f32)
    ET           = Etmp - 1 -> fp8            (DVE)
    rsum[1, i]   = lhsT=ones, rhs=ET[:, kk, i]                (fp8 DR)
    O[i, c]      = lhsT=ET[:, kk, i], rhs=V[:, kk, c]         (fp8 DR)
    out          = (O + colsum_bcast) * (1 / (65536 + 32*rsum))  (DVE)
"""

from contextlib import ExitStack

import ml_dtypes
import numpy as np

import concourse.tile as tile
from concourse import bacc, mybir
from concourse.bass_utils import run_bass_kernel_spmd
from concourse.masks import make_identity

F32 = mybir.dt.float32
BF16 = mybir.dt.bfloat16
F8 = mybir.dt.float8e4
P = 128
DR = mybir.MatmulPerfMode.DoubleRow

NP_F8 = mybir.dt.np(F8)
NP_BF16 = mybir.dt.np(BF16)


def build_attention(nc, S=2048, D=1024, QB=1024, n_cores=8):
    """Emit the per-core attention program into `nc`. Requires S == 2*D == 2*QB."""
    assert S == 2 * D and QB == D and D % P == 0
    NST = S // P        # seq tiles (16)
    NDT = D // P        # d_model tiles (8)
    NQT = QB // P       # query tiles for this core (8)
    NC = 512            # matmul free-dim chunk (one PSUM bank of f32)
    NCH_D = D // NC     # chunks over output channels (2)
    NCH_Q = QB // NC    # chunks over queries (2)
    EXP = mybir.ActivationFunctionType.Exp
    groups = [[2 * b, 2 * b + 1] for b in range(n_cores // 2)]

    xqT_ap = nc.dram_tensor("xqT", [D, QB], F8, kind="ExternalInput").ap()
    xpT_ap = nc.dram_tensor("xpT", [D, D], F8, kind="ExternalInput").ap()
    wqT_ap = nc.dram_tensor("wqT", [D, D], F8, kind="ExternalInput").ap()
    wkT_ap = nc.dram_tensor("wkT", [D, D], F8, kind="ExternalInput").ap()
    wvT_ap = nc.dram_tensor("wvT", [D, D], F8, kind="ExternalInput").ap()
    wvT16_ap = nc.dram_tensor("wvT16", [D, D], BF16, kind="ExternalInput").ap()
    xsT_ap = nc.dram_tensor("xsT", [D, 1], BF16, kind="ExternalInput").ap()
    out_ap = nc.dram_tensor("out", [QB, D], BF16, kind="ExternalOutput").ap()

    with tile.TileContext(nc) as tc, ExitStack() as ctx:
        const_pool = ctx.enter_context(tc.tile_pool(name="const", bufs=1))
        big_pool = ctx.enter_context(tc.tile_pool(name="big", bufs=1))
        dram = ctx.enter_context(tc.tile_pool(name="dram", bufs=1, space="DRAM"))
        psum_mm = ctx.enter_context(tc.tile_pool(name="psum_mm", bufs=4, space="PSUM"))

        ones8 = const_pool.tile([P, NST, 1], F8)
        nc.vector.memset(ones8, 1.0)
        ones16 = const_pool.tile([1, P], BF16)
        nc.vector.memset(ones16, 1.0)
        identity = const_pool.tile([P, P], BF16)
        make_identity(nc, identity)

        # big operand tiles, 3D [P, k_tiles, cols]
        xqT = big_pool.tile([P, NDT, QB], F8, name="xqT_t")
        xpT = big_pool.tile([P, NDT, D], F8, name="xpT_t")
        wqT = big_pool.tile([P, NDT, D], F8, name="wqT_t")
        wkT = big_pool.tile([P, NDT, D], F8, name="wkT_t")
        wvT = big_pool.tile([P, NDT, D], F8, name="wvT_t")
        wvT16 = big_pool.tile([P, NDT, D], BF16, name="wvT16_t")
        xsT = big_pool.tile([P, NDT, 1], BF16, name="xsT_t")
        QT = big_pool.tile([P, NDT, QB], F8, name="QT_t")
        KR = big_pool.tile([P, NDT, S], F8, name="KR_t")
        V = big_pool.tile([P, NST, D], F8, name="V_t")
        ET = big_pool.tile([P, NST, QB], F8, name="ET_t")
        cbc = big_pool.tile([P, D], F32, name="cbc")        # colsum' broadcast
        csrow = big_pool.tile([1, D], BF16, name="csrow")   # colsum' row
        rsrow = big_pool.tile([1, QB], BF16, name="rsrow")  # rowsum(e) row
        rc_all = big_pool.tile([P, NQT], F32, name="rc_all")

        # DRAM bounce buffers for the pair AllGathers
        warm_in = dram.tile([1, P], F8, name="warm_in")
        warm_out = dram.tile([2, P], F8, name="warm_out")
        kr_frag = dram.tile([NDT, P, D], F8, name="kr_frag")
        kr_gath = dram.tile([2, NDT, P, D], F8, name="kr_gath")
        v_frag = dram.tile([NQT, P, D], F8, name="v_frag")
        v_gath = dram.tile([2, NQT, P, D], F8, name="v_gath")

        def load3d(dst3, src_ap, nrt, queue=nc.sync):
            for rt in range(nrt):
                queue.dma_start(
                    out=dst3[:, rt], in_=src_ap[rt * P:(rt + 1) * P, :]
                )

        def mm_chain(pm, lhsT3, lslice, rhs3, rslice, nkt):
            for kt in range(0, nkt, 2):
                nc.tensor.matmul(
                    pm[:],
                    lhsT3[:, kt:kt + 2, lslice],
                    rhs3[:, kt:kt + 2, rslice],
                    start=(kt == 0), stop=(kt == nkt - 2),
                    perf_mode=DR,
                )

        with tc.tile_pool(name="frag", bufs=2) as frag_pool, \
                tc.tile_pool(name="psum_cs", bufs=2, space="PSUM") as psum_cs:

            # ---- Phase A: K_r fragment -> AllGather (launch ASAP) ----
            # tiny warmup collective: absorbs the one-time global barrier and
            # CC-stream setup while the first loads are in flight
            nc.gpsimd.collective_compute(
                "AllGather", mybir.AluOpType.bypass, replica_groups=groups,
                ins=[warm_in.opt()], outs=[warm_out.opt()],
            )
            load3d(wkT, wkT_ap, NDT, queue=nc.sync)
            load3d(xpT, xpT_ap, NDT, queue=nc.scalar)
            for mt in range(NDT):
                kf = frag_pool.tile([P, D], F8, tag="kf", name="kf")
                for cch in range(NCH_D):
                    pm = psum_mm.tile([P, NC], F32, tag="pm")
                    mm_chain(pm, xpT, slice(mt * P, (mt + 1) * P),
                             wkT, slice(cch * NC, (cch + 1) * NC), NDT)
                    nc.scalar.copy(kf[:, cch * NC:(cch + 1) * NC], pm[:])
                nc.scalar.dma_start(out=kr_frag[mt], in_=kf[:])
            nc.gpsimd.collective_compute(
                "AllGather", mybir.AluOpType.bypass, replica_groups=groups,
                ins=[kr_frag.opt()], outs=[kr_gath.opt()],
            )

            # ---- Phase B: QT, V fragment, colsum (hides KR AllGather) ----
            load3d(xqT, xqT_ap, NDT, queue=nc.sync)
            load3d(wqT, wqT_ap, NDT, queue=nc.scalar)
            load3d(wvT, wvT_ap, NDT, queue=nc.sync)
            load3d(wvT16, wvT16_ap, NDT, queue=nc.scalar)
            for kt in range(NDT):
                nc.sync.dma_start(out=xsT[:, kt], in_=xsT_ap[kt * P:(kt + 1) * P, :])

            for mt in range(NDT):
                for ich in range(NCH_Q):
                    pm = psum_mm.tile([P, NC], F32, tag="pm")
                    mm_chain(pm, wqT, slice(mt * P, (mt + 1) * P),
                             xqT, slice(ich * NC, (ich + 1) * NC), NDT)
                    nc.vector.tensor_copy(QT[:, mt, ich * NC:(ich + 1) * NC], pm[:])

            for st in range(NQT):
                vf = frag_pool.tile([P, D], F8, tag="vf", name="vf")
                for cch in range(NCH_D):
                    pm = psum_mm.tile([P, NC], F32, tag="pm")
                    mm_chain(pm, xqT, slice(st * P, (st + 1) * P),
                             wvT, slice(cch * NC, (cch + 1) * NC), NDT)
                    nc.vector.tensor_copy(vf[:, cch * NC:(cch + 1) * NC], pm[:])
                nc.scalar.dma_start(out=v_frag[st], in_=vf[:])

            # colsum' = xsum @ (32 W_V).T, bf16 (accuracy-critical rank-1 term)
            for cch in range(NCH_D):
                pc = psum_cs.tile([1, NC], F32, tag="pc")
                for kt in range(NDT):
                    nc.tensor.matmul(
                        pc[:], xsT[:, kt, :], wvT16[:, kt, cch * NC:(cch + 1) * NC],
                        start=(kt == 0), stop=(kt == NDT - 1),
                    )
                nc.vector.tensor_copy(csrow[:, cch * NC:(cch + 1) * NC], pc[:])

            nc.gpsimd.collective_compute(
                "AllGather", mybir.AluOpType.bypass, replica_groups=groups,
                ins=[v_frag.opt()], outs=[v_gath.opt()],
            )

            # broadcast colsum' across partitions via K=1 ones matmul
            for cch in range(NCH_D):
                pm = psum_mm.tile([P, NC], F32, tag="pm")
                nc.tensor.matmul(
                    pm[:], ones16[:], csrow[:, cch * NC:(cch + 1) * NC],
                    start=True, stop=True,
                )
                nc.vector.tensor_copy(cbc[:, cch * NC:(cch + 1) * NC], pm[:])

            # pull gathered K_r halves into SBUF (ACT HWDGE queue is idle now)
            for g in range(2):
                for mt in range(NDT):
                    nc.scalar.dma_start(
                        out=KR[:, mt, g * D:(g + 1) * D], in_=kr_gath[g, mt]
                    )

        # ---- Phase C: scores + exp + e-cast; rsum interleaved ----
        with tc.tile_pool(name="estage", bufs=4) as estage, \
                tc.tile_pool(name="ostage", bufs=4) as ostage, \
                tc.tile_pool(name="psum_rs", bufs=1, space="PSUM") as psum_rs, \
                tc.tile_pool(name="psum_t", bufs=1, space="PSUM") as psum_t:

            # V loads (sync queue; behind the V AllGather)
            for g in range(2):
                for st in range(NQT):
                    nc.sync.dma_start(out=V[:, g * NQT + st, :], in_=v_gath[g, st])

            prs = [psum_rs.tile([1, NC], F32, tag=f"prs{i}", name=f"prs{i}")
                   for i in range(NCH_Q)]
            for jt in range(NST):
                for ich in range(NCH_Q):
                    pm = psum_mm.tile([P, NC], F32, tag="pm")
                    mm_chain(pm, KR, slice(jt * P, (jt + 1) * P),
                             QT, slice(ich * NC, (ich + 1) * NC), NDT)
                    et = estage.tile([P, NC], F32, tag="et", name="et")
                    nc.scalar.activation(et[:], pm[:], EXP, scale=1.0 / (1 << 20))
                    nc.vector.tensor_scalar_add(
                        ET[:, jt, ich * NC:(ich + 1) * NC], et[:], -1.0
                    )
                # rsum accumulation (plain fp8 mode: DoubleRow with a
                # 1-partition output fails walrus codegen)
                for ich in range(NCH_Q):
                    nc.tensor.matmul(
                        prs[ich][:],
                        ones8[:, jt, :],
                        ET[:, jt, ich * NC:(ich + 1) * NC],
                        start=(jt == 0), stop=(jt == NST - 1),
                    )

            for ich in range(NCH_Q):
                nc.vector.tensor_copy(rsrow[:, ich * NC:(ich + 1) * NC], prs[ich][:])

            # ---- Phase D: recips + e.T @ V + combine ----
            # "transpose" rsrow [1, QB] into per-partition column [P, NQT] via
            # K=1 matmuls against a [1, 1] ones tile (lhsT free -> partitions)
            rs_t = psum_t.tile([P, NQT], F32, name="rs_t")
            for it in range(NQT):
                nc.tensor.matmul(
                    rs_t[:, it:it + 1], rsrow[:, it * P:(it + 1) * P],
                    ones16[:, 0:1], start=True, stop=True,
                )
            rs32 = const_pool.tile([P, NQT], F32, name="rs32")
            nc.vector.tensor_scalar(
                rs32[:], rs_t[:], 32.0, 65536.0,
                mybir.AluOpType.mult, mybir.AluOpType.add,
            )
            nc.vector.reciprocal(rc_all[:], rs32[:])

            for it in range(NQT):
                for cch in range(NCH_D):
                    pm = psum_mm.tile([P, NC], F32, tag="pm")
                    mm_chain(pm, ET, slice(it * P, (it + 1) * P),
                             V, slice(cch * NC, (cch + 1) * NC), NST)
                    ob = ostage.tile([P, NC], F32, tag="ob", name="ob")
                    nc.vector.tensor_add(ob[:], pm[:], cbc[:, cch * NC:(cch + 1) * NC])
                    ob16 = ostage.tile([P, NC], BF16, tag="ob16", name="ob16")
                    nc.vector.tensor_scalar_mul(ob16[:], ob[:], rc_all[:, it:it + 1])
                    (nc.sync if (it + cch) % 2 == 0 else nc.scalar).dma_start(
                        out=out_ap[it * P:(it + 1) * P, cch * NC:(cch + 1) * NC],
                        in_=ob16[:],
                    )
    return nc


_CACHE = {}


def _get_nc(S=2048, D=1024, QB=1024):
    key = (S, D, QB)
    if key not in _CACHE:
        nc = bacc.Bacc("TRN2", target_bir_lowering=False, debug=False, num_devices=8)
        build_attention(nc, S=S, D=D, QB=QB, n_cores=8)
        nc.compile()
        _CACHE[key] = nc
    return _CACHE[key]


def _run(x, W_Q, W_K, W_V, **spmd_kwargs):
    B, S, D = x.shape  # (4, 2048, 1024)
    QB = S // 2        # queries per core (1024)
    x = np.asarray(x, dtype=np.float32)
    W_Q = np.asarray(W_Q, dtype=np.float32)
    W_K = np.asarray(W_K, dtype=np.float32)
    W_V = np.asarray(W_V, dtype=np.float32)

    # weights pre-scaled by 32 so elements (~N(0, 1/D)) use fp8's normal range
    wqT8 = np.ascontiguousarray(W_Q.T * 32.0).astype(NP_F8)
    wkT8 = np.ascontiguousarray(W_K.T * 32.0).astype(NP_F8)
    wvT8 = np.ascontiguousarray(W_V.T * 32.0).astype(NP_F8)
    wvT16 = np.ascontiguousarray(W_V.T * 32.0).astype(NP_BF16)

    nc = _get_nc(S=S, D=D, QB=QB)
    in_maps = []
    for core in range(8):
        b, h = core // 2, core % 2
        xTb = x[b].T  # [D, S] view
        in_maps.append({
            "xqT": np.ascontiguousarray(xTb[:, h * QB:(h + 1) * QB]).astype(NP_F8),
            "xpT": np.ascontiguousarray(xTb[:, h::2]).astype(NP_F8),
            "wqT": wqT8, "wkT": wkT8, "wvT": wvT8, "wvT16": wvT16,
            "xsT": x[b].sum(axis=0, dtype=np.float64).astype(NP_BF16).reshape(D, 1),
        })
    res = run_bass_kernel_spmd(nc, in_maps, list(range(8)), **spmd_kwargs)
    out = np.empty((B, S, D), dtype=np.float32)
    for core in range(8):
        b, h = core // 2, core % 2
        out[b, h * QB:(h + 1) * QB, :] = np.asarray(
            res.results[core]["out"]).astype(np.float32)
    return out, res


def kernel(x, W_Q, W_K, W_V):
    return _run(x, W_Q, W_K, W_V)[0]


# revision 13
# speedup vs baseline: 1.1311x; 1.0226x over previous
"""Trainium2 Bass kernel for single-head attention with row-major K-reshape.

Reference computation (per batch b):
    Q = x @ W_Q.T ; K = x @ W_K.T ; V = x @ W_V.T          # [S, D]
    K_r = K.reshape(D, S)          # row-major reshape, NOT a transpose
    scores = Q @ K_r / D
    out = softmax(scores, -1) @ V
Shapes: B=4, S=2048, D=1024, f32.

Sharding: 8 cores = (batch b in 0..3) x (pair-rank h in 0..1).  Core (b, h)
computes out[b, h*QB:(h+1)*QB, :].  K_r / V are built cooperatively by the
pair and exchanged with 2-rank AllGathers (with S == 2*D the row-major
reshape gives K_r[m, g*D + c] = K[2m + g, c], so rank g's K_r half is
x[g::2] @ W_K.T and its V half is its own query rows xq @ W_V.T).

Numerics: all five big matmuls run in fp8(e4m3) with DoubleRow perf mode
(2 contraction rows / PE cycle).  fp8 is safe here because:
  - scores are tiny (std ~1/32), so fp8 Q/K error feeds softmax as a ~1e-3
    absolute score perturbation -> negligible.
  - E = exp(scores) ~= 1, so we materialize e = E - 1 (small, fp8-safe) and
    use  softmax @ V = (colsum(V) + e.T @ V) / rsum,  with colsum(V) =
    (sum_rows x) @ W_V.T computed separately in bf16, folded in on the DVE
    drain, and rsum = S + rowsum(e).
  - weights are pre-scaled by 32 on the host so their elements (~N(0,1/D))
    land in fp8's normal range; the scale is folded into the exp scale and
    the final reciprocal.

The host passes pre-transposed operands (xT slices, W.T) so the device does
zero transposes of the inputs (PE transposes measured ~13x theoretical cost).

Per-core matmul dataflow (TensorE: out[M,N] = lhsT[K,M].T @ rhs[K,N],
contraction over the partition dim; all operand tiles are 3D
[P, k_tiles, cols] so DoubleRow can consume k-tile pairs):
    QT[m, i]     = lhsT=wqT[:, kk, m],  rhs=xqT[:, kk, i]     (fp8 DR)
    KRfrag[m, c] = lhsT=xpT[:, kk, m],  rhs=wkT[:, kk, c]     (fp8 DR)
    Vfrag[s', c] = lhsT=xqT[:, kk, s'], rhs=wvT[:, kk, c]     (fp8 DR)
    KR / V       = pair AllGather of fragments (DRAM bounce, fp8)
    ST[j, i]     = lhsT=KR[:, kk, j],   rhs=QT[:, kk, i]      (fp8 DR)
    Etmp         = exp(ST * 2^-20)            (ACT, psum->sbuf f32)
    ET           = Etmp - 1 -> fp8            (DVE)
    rsum[1, i]   = lhsT=ones, rhs=ET[:, kk, i]                (fp8 DR)
    O[i, c]      = lhsT=ET[:, kk, i], rhs=V[:, kk, c]         (fp8 DR)
    out          = (O + colsum_bcast) * (1 / (65536 + 32*rsum))  (DVE)
"""

from contextlib import ExitStack

import ml_dtypes
import numpy as np

import concourse.tile as tile
from concourse import bacc, mybir
from concourse.bass_utils import run_bass_kernel_spmd
from concourse.masks import make_identity

F32 = mybir.dt.float32
BF16 = mybir.dt.bfloat16
F8 = mybir.dt.float8e4
P = 128
DR = mybir.MatmulPerfMode.DoubleRow

NP_F8 = mybir.dt.np(F8)
NP_BF16 = mybir.dt.np(BF16)


def build_attention(nc, S=2048, D=1024, QB=1024, n_cores=8):
    """Emit the per-core attention program into `nc`. Requires S == 2*D == 2*QB."""
    assert S == 2 * D and QB == D and D % P == 0
    NST = S // P        # seq tiles (16)
    NDT = D // P        # d_model tiles (8)
    NQT = QB // P       # query tiles for this core (8)
    NC = 512            # matmul free-dim chunk (one PSUM bank of f32)
    NCH_D = D // NC     # chunks over output channels (2)
    NCH_Q = QB // NC    # chunks over queries (2)
    EXP = mybir.ActivationFunctionType.Exp
    groups = [[2 * b, 2 * b + 1] for b in range(n_cores // 2)]

    xqT_ap = nc.dram_tensor("xqT", [D, QB], F8, kind="ExternalInput").ap()
    xpT_ap = nc.dram_tensor("xpT", [D, D], F8, kind="ExternalInput").ap()
    wqT_ap = nc.dram_tensor("wqT", [D, D], F8, kind="ExternalInput").ap()
    wkT_ap = nc.dram_tensor("wkT", [D, D], F8, kind="ExternalInput").ap()
    wvT_ap = nc.dram_tensor("wvT", [D, D], F8, kind="ExternalInput").ap()
    wvT16_ap = nc.dram_tensor("wvT16", [D, D], BF16, kind="ExternalInput").ap()
    xsT_ap = nc.dram_tensor("xsT", [D, 1], BF16, kind="ExternalInput").ap()
    out_ap = nc.dram_tensor("out", [QB, D], BF16, kind="ExternalOutput").ap()

    with tile.TileContext(nc) as tc, ExitStack() as ctx:
        const_pool = ctx.enter_context(tc.tile_pool(name="const", bufs=1))
        big_pool = ctx.enter_context(tc.tile_pool(name="big", bufs=1))
        dram = ctx.enter_context(tc.tile_pool(name="dram", bufs=1, space="DRAM"))
        psum_mm = ctx.enter_context(tc.tile_pool(name="psum_mm", bufs=4, space="PSUM"))

        ones8 = const_pool.tile([P, NST, 1], F8)
        nc.vector.memset(ones8, 1.0)
        ones16 = const_pool.tile([1, P], BF16)
        nc.vector.memset(ones16, 1.0)
        identity = const_pool.tile([P, P], BF16)
        make_identity(nc, identity)

        # big operand tiles, 3D [P, k_tiles, cols]
        xqT = big_pool.tile([P, NDT, QB], F8, name="xqT_t")
        xpT = big_pool.tile([P, NDT, D], F8, name="xpT_t")
        wqT = big_pool.tile([P, NDT, D], F8, name="wqT_t")
        wkT = big_pool.tile([P, NDT, D], F8, name="wkT_t")
        wvT = big_pool.tile([P, NDT, D], F8, name="wvT_t")
        wvT16 = big_pool.tile([P, NDT, D], BF16, name="wvT16_t")
        xsT = big_pool.tile([P, NDT, 1], BF16, name="xsT_t")
        QT = big_pool.tile([P, NDT, QB], F8, name="QT_t")
        KR = big_pool.tile([P, NDT, S], F8, name="KR_t")
        V = big_pool.tile([P, NST, D], F8, name="V_t")
        ET = big_pool.tile([P, NST, QB], F8, name="ET_t")
        cbc = big_pool.tile([P, D], F32, name="cbc")        # colsum' broadcast
        csrow = big_pool.tile([1, D], BF16, name="csrow")   # colsum' row
        rsrow = big_pool.tile([1, QB], BF16, name="rsrow")  # rowsum(e) row
        rc_all = big_pool.tile([P, NQT], F32, name="rc_all")

        # DRAM bounce buffers for the pair AllGathers
        kr_frag = dram.tile([NDT, P, D], F8, name="kr_frag")
        kr_gath = dram.tile([2, NDT, P, D], F8, name="kr_gath")
        v_frag = dram.tile([NQT, P, D], F8, name="v_frag")
        v_gath = dram.tile([2, NQT, P, D], F8, name="v_gath")

        def load3d(dst3, src_ap, nrt, queue=nc.sync):
            for rt in range(nrt):
                queue.dma_start(
                    out=dst3[:, rt], in_=src_ap[rt * P:(rt + 1) * P, :]
                )

        def mm_chain(pm, lhsT3, lslice, rhs3, rslice, nkt):
            for kt in range(0, nkt, 2):
                nc.tensor.matmul(
                    pm[:],
                    lhsT3[:, kt:kt + 2, lslice],
                    rhs3[:, kt:kt + 2, rslice],
                    start=(kt == 0), stop=(kt == nkt - 2),
                    perf_mode=DR,
                )

        with tc.tile_pool(name="frag", bufs=2) as frag_pool, \
                tc.tile_pool(name="psum_cs", bufs=2, space="PSUM") as psum_cs:

            # ---- Phase A: K_r fragment -> AllGather (launch ASAP) ----
            load3d(wkT, wkT_ap, NDT, queue=nc.sync)
            load3d(xpT, xpT_ap, NDT, queue=nc.scalar)
            for mt in range(NDT):
                kf = frag_pool.tile([P, D], F8, tag="kf", name="kf")
                for cch in range(NCH_D):
                    pm = psum_mm.tile([P, NC], F32, tag="pm")
                    mm_chain(pm, xpT, slice(mt * P, (mt + 1) * P),
                             wkT, slice(cch * NC, (cch + 1) * NC), NDT)
                    nc.scalar.copy(kf[:, cch * NC:(cch + 1) * NC], pm[:])
                nc.gpsimd.dma_start(out=kr_frag[mt], in_=kf[:])
            nc.gpsimd.collective_compute(
                "AllGather", mybir.AluOpType.bypass, replica_groups=groups,
                ins=[kr_frag.opt()], outs=[kr_gath.opt()],
            )

            # ---- Phase B: QT, V fragment, colsum (hides KR AllGather) ----
            load3d(xqT, xqT_ap, NDT, queue=nc.sync)
            load3d(wqT, wqT_ap, NDT, queue=nc.scalar)
            load3d(wvT, wvT_ap, NDT, queue=nc.sync)
            load3d(wvT16, wvT16_ap, NDT, queue=nc.scalar)
            for kt in range(NDT):
                nc.sync.dma_start(out=xsT[:, kt], in_=xsT_ap[kt * P:(kt + 1) * P, :])

            for mt in range(NDT):
                for ich in range(NCH_Q):
                    pm = psum_mm.tile([P, NC], F32, tag="pm")
                    mm_chain(pm, wqT, slice(mt * P, (mt + 1) * P),
                             xqT, slice(ich * NC, (ich + 1) * NC), NDT)
                    nc.vector.tensor_copy(QT[:, mt, ich * NC:(ich + 1) * NC], pm[:])

            for st in range(NQT):
                vf = frag_pool.tile([P, D], F8, tag="vf", name="vf")
                for cch in range(NCH_D):
                    pm = psum_mm.tile([P, NC], F32, tag="pm")
                    mm_chain(pm, xqT, slice(st * P, (st + 1) * P),
                             wvT, slice(cch * NC, (cch + 1) * NC), NDT)
                    nc.vector.tensor_copy(vf[:, cch * NC:(cch + 1) * NC], pm[:])
                nc.gpsimd.dma_start(out=v_frag[st], in_=vf[:])

            # colsum' = xsum @ (32 W_V).T, bf16 (accuracy-critical rank-1 term)
            for cch in range(NCH_D):
                pc = psum_cs.tile([1, NC], F32, tag="pc")
                for kt in range(NDT):
                    nc.tensor.matmul(
                        pc[:], xsT[:, kt, :], wvT16[:, kt, cch * NC:(cch + 1) * NC],
                        start=(kt == 0), stop=(kt == NDT - 1),
                    )
                nc.vector.tensor_copy(csrow[:, cch * NC:(cch + 1) * NC], pc[:])

            nc.gpsimd.collective_compute(
                "AllGather", mybir.AluOpType.bypass, replica_groups=groups,
                ins=[v_frag.opt()], outs=[v_gath.opt()],
            )

            # broadcast colsum' across partitions via K=1 ones matmul
            for cch in range(NCH_D):
                pm = psum_mm.tile([P, NC], F32, tag="pm")
                nc.tensor.matmul(
                    pm[:], ones16[:], csrow[:, cch * NC:(cch + 1) * NC],
                    start=True, stop=True,
                )
                nc.vector.tensor_copy(cbc[:, cch * NC:(cch + 1) * NC], pm[:])

            # pull gathered K_r halves into SBUF (ACT HWDGE queue is idle now)
            for g in range(2):
                for mt in range(NDT):
                    (nc.scalar if g == 0 else nc.sync).dma_start(
                        out=KR[:, mt, g * D:(g + 1) * D], in_=kr_gath[g, mt]
                    )

        # ---- Phase C: scores + exp + e-cast; rsum interleaved ----
        with tc.tile_pool(name="estage", bufs=4) as estage, \
                tc.tile_pool(name="ostage", bufs=4) as ostage, \
                tc.tile_pool(name="psum_rs", bufs=1, space="PSUM") as psum_rs, \
                tc.tile_pool(name="psum_t", bufs=1, space="PSUM") as psum_t:

            # V loads (sync queue; behind the V AllGather)
            for g in range(2):
                for st in range(NQT):
                    nc.sync.dma_start(out=V[:, g * NQT + st, :], in_=v_gath[g, st])

            prs = [psum_rs.tile([1, NC], F32, tag=f"prs{i}", name=f"prs{i}")
                   for i in range(NCH_Q)]
            for jt in range(NST):
                for ich in range(NCH_Q):
                    pm = psum_mm.tile([P, NC], F32, tag="pm")
                    mm_chain(pm, KR, slice(jt * P, (jt + 1) * P),
                             QT, slice(ich * NC, (ich + 1) * NC), NDT)
                    et = estage.tile([P, NC], F32, tag="et", name="et")
                    nc.scalar.activation(et[:], pm[:], EXP, scale=1.0 / (1 << 20))
                    nc.vector.tensor_scalar_add(
                        ET[:, jt, ich * NC:(ich + 1) * NC], et[:], -1.0
                    )
                # rsum accumulation (plain fp8 mode: DoubleRow with a
                # 1-partition output fails walrus codegen)
                for ich in range(NCH_Q):
                    nc.tensor.matmul(
                        prs[ich][:],
                        ones8[:, jt, :],
                        ET[:, jt, ich * NC:(ich + 1) * NC],
                        start=(jt == 0), stop=(jt == NST - 1),
                    )

            for ich in range(NCH_Q):
                nc.vector.tensor_copy(rsrow[:, ich * NC:(ich + 1) * NC], prs[ich][:])

            # ---- Phase D: recips + e.T @ V + combine ----
            # "transpose" rsrow [1, QB] into per-partition column [P, NQT] via
            # K=1 matmuls against a [1, 1] ones tile (lhsT free -> partitions)
            rs_t = psum_t.tile([P, NQT], F32, name="rs_t")
            for it in range(NQT):
                nc.tensor.matmul(
                    rs_t[:, it:it + 1], rsrow[:, it * P:(it + 1) * P],
                    ones16[:, 0:1], start=True, stop=True,
                )
            rs32 = const_pool.tile([P, NQT], F32, name="rs32")
            nc.vector.tensor_scalar(
                rs32[:], rs_t[:], 32.0, 65536.0,
                mybir.AluOpType.mult, mybir.AluOpType.add,
            )
            nc.vector.reciprocal(rc_all[:], rs32[:])

            for it in range(NQT):
                for cch in range(NCH_D):
                    pm = psum_mm.tile([P, NC], F32, tag="pm")
                    mm_chain(pm, ET, slice(it * P, (it + 1) * P),
                             V, slice(cch * NC, (cch + 1) * NC), NST)
                    ob = ostage.tile([P, NC], F32, tag="ob", name="ob")
                    nc.vector.tensor_add(ob[:], pm[:], cbc[:, cch * NC:(cch + 1) * NC])
                    ob16 = ostage.tile([P, NC], BF16, tag="ob16", name="ob16")
                    nc.vector.tensor_scalar_mul(ob16[:], ob[:], rc_all[:, it:it + 1])
                    (nc.sync if (it + cch) % 2 == 0 else nc.scalar).dma_start(
                        out=out_ap[it * P:(it + 1) * P, cch * NC:(cch + 1) * NC],
                        in_=ob16[:],
                    )
    return nc


_CACHE = {}


def _get_nc(S=2048, D=1024, QB=1024):
    key = (S, D, QB)
    if key not in _CACHE:
        nc = bacc.Bacc("TRN2", target_bir_lowering=False, debug=False, num_devices=8)
        build_attention(nc, S=S, D=D, QB=QB, n_cores=8)
        nc.compile()
        _CACHE[key] = nc
    return _CACHE[key]


def _run(x, W_Q, W_K, W_V, **spmd_kwargs):
    B, S, D = x.shape  # (4, 2048, 1024)
    QB = S // 2        # queries per core (1024)
    x = np.asarray(x, dtype=np.float32)
    W_Q = np.asarray(W_Q, dtype=np.float32)
    W_K = np.asarray(W_K, dtype=np.float32)
    W_V = np.asarray(W_V, dtype=np.float32)

    # weights pre-scaled by 32 so elements (~N(0, 1/D)) use fp8's normal range
    wqT8 = np.ascontiguousarray(W_Q.T * 32.0).astype(NP_F8)
    wkT8 = np.ascontiguousarray(W_K.T * 32.0).astype(NP_F8)
    wvT8 = np.ascontiguousarray(W_V.T * 32.0).astype(NP_F8)
    wvT16 = np.ascontiguousarray(W_V.T * 32.0).astype(NP_BF16)

    nc = _get_nc(S=S, D=D, QB=QB)
    in_maps = []
    for core in range(8):
        b, h = core // 2, core % 2
        xTb = x[b].T  # [D, S] view
        in_maps.append({
            "xqT": np.ascontiguousarray(xTb[:, h * QB:(h + 1) * QB]).astype(NP_F8),
            "xpT": np.ascontiguousarray(xTb[:, h::2]).astype(NP_F8),
            "wqT": wqT8, "wkT": wkT8, "wvT": wvT8, "wvT16": wvT16,
            "xsT": x[b].sum(axis=0, dtype=np.float64).astype(NP_BF16).reshape(D, 1),
        })
    res = run_bass_kernel_spmd(nc, in_maps, list(range(8)), **spmd_kwargs)
    out = np.empty((B, S, D), dtype=np.float32)
    for core in range(8):
        b, h = core // 2, core % 2
        out[b, h * QB:(h + 1) * QB, :] = np.asarray(
            res.results[core]["out"]).astype(np.float32)
    return out, res


def kernel(x, W_Q, W_K, W_V):
    return _run(x, W_Q, W_K, W_V)[0]
